# revision 35
# baseline (speedup 1.0000x reference)
"""Devign GGNN model on 8 Trainium2 NeuronCores (Bass/Tile) — v3.

Strategy (data-parallel over dst-node shards, aggregate-then-transform):
  - 8 cores, core c owns nodes [c*4096, (c+1)*4096).
  - Node state kept transposed on-chip: hT [256 feat, 4096 nodes] bf16.
  - Step 0's aggregated input aT0 is precomputed on the host (h0 is just
    the padded input features), so the device loop starts at the GRU.
  - Per GGNN step s >= 1:
      1. transpose the local h shard to row-major (PE transpose) and DMA
         to a DRAM bounce buffer [4096, 256] fp8,
      2. AllGather -> full RAW node table h [32768, 256] fp8 (the
         per-etype W_msg transform is applied AFTER aggregation, so the
         table is 4x smaller than transformed-message approaches),
      3. edges (sorted by (dst-window, etype), chunk slots quantized to
         64) are processed: indirect-DMA row gather of h[src], then
         one-hot scatter matmuls A_tT[w] += h_chunk^T @ S_chunk (fp8
         DoubleRow pairs two 128-slot chunks per instruction; 64-slot
         tail chunks use partition-sliced matmuls).  S is 0/1 fp8,
         persistent in SBUF.
      4. aT[w] = sum_t W_msg[t]^T @ A_tT  (16 matmuls per window),
      5. GRU cell evaluated per 128-node window in transposed layout.
  - Readout (conv1d/maxpool stacks + gated sum) per graph with convs as
    tap-shifted matmuls.

All index/one-hot/weight-layout preprocessing is done on the host at
kernel() time and baked into the compiled program + per-core inputs.
"""
import sys
import numpy as np

for _p in ("/opt/trn_rl_repo",):
    if _p not in sys.path:
        sys.path.insert(0, _p)

import ml_dtypes

import concourse.bass as bass
import concourse.mybir as mybir
import concourse.tile as tile
from concourse import bacc
from concourse.bass_utils import run_bass_kernel_spmd

BF16 = ml_dtypes.bfloat16
F32 = np.float32

NCORES = 8
NN = 32768          # total nodes
IN_DIM = 128
OUT = 256
NT = 4              # edge types
NSTEPS = 8
NGRAPH = 128
NPC = NN // NCORES  # nodes per core = 4096
WIN = 128           # dst window size
NWIN = NPC // WIN   # 32 windows per core
GPC = NGRAPH // NCORES  # graphs per core = 16
LG = 256            # nodes per graph
CONCAT = IN_DIM + OUT  # 384
GCH = 8             # max chunk-columns per gather call (1024 rows)

bf = mybir.dt.bfloat16
f32 = mybir.dt.float32
fp8 = mybir.dt.float8e4
i16 = mybir.dt.int16
AF = mybir.ActivationFunctionType
ALU = mybir.AluOpType
FP8NP = mybir.dt.np(fp8)


# ---------------------------------------------------------------------------
# weight/bias image layout (shared between host packer and device slicer)
# ---------------------------------------------------------------------------
class WLayout:
    def __init__(self):
        self.col = 0
        self.off = {}
        self.width = {}

    def alloc(self, name, width):
        self.off[name] = self.col
        self.width[name] = width
        self.col += width
        return self.off[name]


def _make_wlayout():
    wl = WLayout()
    for t in range(NT):
        for kc in range(2):
            for mo in range(2):
                wl.alloc(f"wmsg_{t}_{kc}_{mo}", 128)   # lhsT blocks of W_msg[t]
    for g in range(6):
        for kc in range(2):
            wl.alloc(f"wi_{g}_{kc}", 128)              # lhsT blocks
            wl.alloc(f"wh_{g}_{kc}", 128)
    for tap in range(3):
        for kc in range(2):
            for mo in range(2):
                wl.alloc(f"c1_{tap}_{kc}_{mo}", 128)
    for kc in range(2):
        for mo in range(2):
            wl.alloc(f"c2_{kc}_{mo}", 128)
    for tap in range(3):
        for kc in range(3):
            for mo in range(3):
                wl.alloc(f"cc1_{tap}_{kc}_{mo}", 128)
    for kc in range(3):
        for mo in range(3):
            wl.alloc(f"cc2_{kc}_{mo}", 128)
    for kc in range(2):
        wl.alloc(f"wy_{kc}", 1)
    for kc in range(3):
        wl.alloc(f"wz_{kc}", 1)
    wl.alloc("ident", 128)
    return wl


def _make_blayout():
    bl = WLayout()
    for name in ("br", "bz", "big", "bhg"):
        bl.alloc(name, 2)       # [256] as two [128] cols
    bl.alloc("c1b", 2)
    bl.alloc("c2b", 2)
    bl.alloc("cc1b", 3)
    bl.alloc("cc2b", 3)
    bl.alloc("by", 1)
    bl.alloc("bz_", 1)
    return bl


def _pack_weights(wl, W_msg, gru_Wi, gru_Wh, conv1_w, conv2_w, convc1_w, convc2_w, wy, wz):
    img = np.zeros((128, wl.col), np.float32)

    def put(name, block):
        o = wl.off[name]
        img[:, o:o + block.shape[1]] = block

    for t in range(NT):
        for kc in range(2):
            for mo in range(2):
                put(f"wmsg_{t}_{kc}_{mo}",
                    W_msg[t][kc * 128:(kc + 1) * 128, mo * 128:(mo + 1) * 128])
    for g in range(6):
        for kc in range(2):
            put(f"wi_{g}_{kc}", gru_Wi[kc * 128:(kc + 1) * 128, g * 128:(g + 1) * 128])
            put(f"wh_{g}_{kc}", gru_Wh[kc * 128:(kc + 1) * 128, g * 128:(g + 1) * 128])
    for tap in range(3):
        w_t = conv1_w[:, :, tap].T  # [i, o]
        for kc in range(2):
            for mo in range(2):
                put(f"c1_{tap}_{kc}_{mo}", w_t[kc * 128:(kc + 1) * 128, mo * 128:(mo + 1) * 128])
    w2 = conv2_w[:, :, 0].T
    for kc in range(2):
        for mo in range(2):
            put(f"c2_{kc}_{mo}", w2[kc * 128:(kc + 1) * 128, mo * 128:(mo + 1) * 128])
    for tap in range(3):
        w_t = convc1_w[:, :, tap].T
        for kc in range(3):
            for mo in range(3):
                put(f"cc1_{tap}_{kc}_{mo}", w_t[kc * 128:(kc + 1) * 128, mo * 128:(mo + 1) * 128])
    wc2 = convc2_w[:, :, 0].T
    for kc in range(3):
        for mo in range(3):
            put(f"cc2_{kc}_{mo}", wc2[kc * 128:(kc + 1) * 128, mo * 128:(mo + 1) * 128])
    for kc in range(2):
        put(f"wy_{kc}", wy[kc * 128:(kc + 1) * 128, :])
    for kc in range(3):
        put(f"wz_{kc}", wz[kc * 128:(kc + 1) * 128, :])
    put("ident", np.eye(128, dtype=np.float32))
    return img.astype(BF16)


def _pack_biases(bl, gru_bi, gru_bh, conv1_b, conv2_b, convc1_b, convc2_b, by, bz_):
    img = np.zeros((128, bl.col), np.float32)

    def put(name, vec, nch):
        o = bl.off[name]
        for c in range(nch):
            img[:, o + c] = vec[c * 128:(c + 1) * 128]

    put("br", gru_bi[0:256] + gru_bh[0:256], 2)
    put("bz", gru_bi[256:512] + gru_bh[256:512], 2)
    put("big", gru_bi[512:768], 2)
    put("bhg", gru_bh[512:768], 2)
    put("c1b", conv1_b, 2)
    put("c2b", conv2_b, 2)
    put("cc1b", convc1_b, 3)
    put("cc2b", convc2_b, 3)
    img[0, bl.off["by"]] = by[0]
    img[0, bl.off["bz_"]] = bz_[0]
    return img


# ---------------------------------------------------------------------------
# edge preprocessing: sort by (dst-window, etype), 64-quantized chunk slots
# ---------------------------------------------------------------------------
# Chunk plan per (window, etype): F[t] full 128-slot columns + optional
# 64-slot tail.  Tails of different etypes are packed two per column
# (partitions 0:64 / 64:128).  The plan (F, tails, column layout) is uniform
# across cores/windows so the compiled program structure is shared.

def _chunk_plan(C64):
    C64 = [max(2, int(c)) for c in C64]
    F = [c // 2 for c in C64]
    tl = [c % 2 for c in C64]
    colof = [0]
    for t in range(NT):
        colof.append(colof[-1] + F[t])
    ncf = colof[-1]
    tails = [t for t in range(NT) if tl[t]]
    tailpos = {}
    for i, t in enumerate(tails):
        tailpos[t] = (ncf + i // 2, 64 * (i % 2))
    SCF = ncf + (len(tails) + 1) // 2
    return F, tl, colof, tailpos, SCF


def _preprocess_edges(src, dst, etype):
    core = dst // NPC
    w = (dst % NPC) // WIN
    dloc = (dst % WIN).astype(np.int64)
    key = ((core * NWIN + w) * NT + etype).astype(np.int64)
    order = np.argsort(key, kind="stable")
    cnt = np.bincount(key, minlength=NCORES * NWIN * NT).reshape(NCORES, NWIN, NT)
    C64 = np.maximum(1, -(-cnt // 64)).max(axis=(0, 1)).astype(int)  # [NT]
    F, tl, colof, tailpos, SCF = _chunk_plan(C64)
    TOTCH = NWIN * SCF
    starts = np.zeros(NCORES * NWIN * NT + 1, np.int64)
    starts[1:] = np.cumsum(cnt.reshape(-1))
    assert int(src.max()) < 32768

    idx16 = np.zeros((NCORES, 128, TOTCH * 8), np.int16)
    S_img = np.zeros((NCORES, 128, TOTCH * 128), FP8NP)
    for c in range(NCORES):
        flat = np.zeros(TOTCH * 128, np.int32)
        Sc = np.zeros((128, TOTCH * 128), np.float32)
        for wdx in range(NWIN):
            wbase = wdx * SCF
            for t in range(NT):
                b = (c * NWIN + wdx) * NT + t
                seg = order[starts[b]:starts[b + 1]]
                li = src[seg].astype(np.int32)
                dl = dloc[seg]
                nfull = min(len(seg), F[t] * 128)
                for j in range(F[t]):
                    col = wbase + colof[t] + j
                    lo, hi = j * 128, min(nfull, (j + 1) * 128)
                    if hi > lo:
                        flat[col * 128:col * 128 + (hi - lo)] = li[lo:hi]
                        Sc[np.arange(hi - lo), col * 128 + dl[lo:hi]] = 1.0
                rest = len(seg) - nfull
                assert rest <= (64 if tl[t] else 0), (rest, t)
                if tl[t] and rest > 0:
                    tc_, toff = tailpos[t]
                    col = wbase + tc_
                    flat[col * 128 + toff:col * 128 + toff + rest] = li[nfull:]
                    Sc[toff + np.arange(rest), col * 128 + dl[nfull:]] = 1.0
        wrapped = flat.astype(np.int16).reshape(TOTCH * 8, 16).T
        idx16[c] = np.tile(wrapped, (8, 1))
        S_img[c] = Sc.astype(FP8NP)
    return idx16, S_img, C64, SCF


def _precompute_step0(features, src, dst, etype, W_msg):
    """aT for step 0 (h0 = padded features), in the on-chip transposed
    layout: img[c][p, w*256 + kc*128 + d] = aT0[c*NPC + w*128 + d, kc*128+p]."""
    h0 = np.zeros((NN, OUT), np.float32)
    h0[:, :IN_DIM] = features
    aT0 = np.zeros((NN, OUT), np.float32)
    for t in range(NT):
        m = etype == t
        acc = np.zeros((NN, OUT), np.float32)
        np.add.at(acc, dst[m], h0[src[m]])
        aT0 += acc @ W_msg[t]
    out = []
    for c in range(NCORES):
        # cols = wp*512 + kc*256 + wi2*128 + d
        a = aT0[c * NPC:(c + 1) * NPC].reshape(NWIN // 2, 2, 128, 2, 128)
        img = a.transpose(4, 0, 3, 1, 2).reshape(128, NWIN * 256)
        out.append(np.ascontiguousarray(img).astype(BF16))
    return out


# ---------------------------------------------------------------------------
# device program
# ---------------------------------------------------------------------------
def build_program(C64, wl, bl, num_devices=NCORES, sim_mode=False, nsteps=NSTEPS,
                  ablate=(), table_fp8=True, nq=4, single_packet=True):
    ablate = set(ablate)
    tdt = fp8 if table_fp8 else bf
    F, tl, colof, tailpos, SCF = _chunk_plan(C64)
    TOTCH = NWIN * SCF
    NCALL = -(-SCF // GCH)           # gather calls per window
    nc = bacc.Bacc("TRN2", target_bir_lowering=False, debug=False,
                   num_devices=num_devices, num_swdge_queues=nq)
    h0T_d = nc.dram_tensor("h0T", [OUT, NPC], bf, kind="ExternalInput")
    xT_d = nc.dram_tensor("xT", [IN_DIM, NPC], bf, kind="ExternalInput")
    aT0_d = nc.dram_tensor("aT0", [128, NWIN * 256], bf, kind="ExternalInput")
    Wimg_d = nc.dram_tensor("Wimg", [128, wl.col], bf, kind="ExternalInput")
    Bimg_d = nc.dram_tensor("Bimg", [128, bl.col], f32, kind="ExternalInput")
    idx_d = nc.dram_tensor("idx", [128, TOTCH * 8], i16, kind="ExternalInput")
    Simg_d = nc.dram_tensor("Simg", [128, TOTCH * 128], fp8, kind="ExternalInput")
    out_d = nc.dram_tensor("out", [GPC], f32, kind="ExternalOutput")

    with tile.TileContext(nc) as tc:
        with tc.tile_pool(name="persist", bufs=1) as pp, \
             tc.tile_pool(name="dram", bufs=1, space="DRAM") as dpool:
            hT0 = pp.tile([128, NPC], bf)
            hT1 = pp.tile([128, NPC], bf)
            aT0sb = pp.tile([128, NWIN * 256], bf)
            Wsb = pp.tile([128, wl.col], bf)
            Bsb = pp.tile([128, bl.col], f32)
            idx_sb = pp.tile([128, TOTCH * 8], i16)
            Ssb = pp.tile([128, TOTCH * 128], fp8)
            nc.sync.dma_start(hT0[:], h0T_d.ap()[0:128, :])
            nc.sync.dma_start(hT1[:], h0T_d.ap()[128:256, :])
            nc.sync.dma_start(aT0sb[:], aT0_d.ap())
            nc.sync.dma_start(Wsb[:], Wimg_d.ap())
            nc.sync.dma_start(Bsb[:], Bimg_d.ap())
            nc.sync.dma_start(idx_sb[:], idx_d.ap())
            nc.sync.dma_start(Ssb[:], Simg_d.ap())
            hT = [hT0, hT1]

            def W(name):
                o = wl.off[name]
                return Wsb[:, o:o + wl.width[name]]

            def Bias(name, c=0):
                o = bl.off[name] + c
                return Bsb[:, o:o + 1]

            def Bias1(name):
                o = bl.off[name]
                return Bsb[0:1, o:o + 1]

            # ---------------- GGNN loop ----------------
            gdum = None
            if "gather" in ablate:
                gdum = [pp.tile([128, GCH, OUT], tdt, name="gdum0")]
                nc.gpsimd.memset(gdum[0][:], 0.0)
            with tc.tile_pool(name="stage", bufs=3) as stp, \
                 tc.tile_pool(name="gat", bufs=3) as gap, \
                 tc.tile_pool(name="asb", bufs=2) as asp, \
                 tc.tile_pool(name="gtmp", bufs=2) as gtp, \
                 tc.tile_pool(name="pT", bufs=1, space="PSUM") as pT, \
                 tc.tile_pool(name="pA", bufs=1, space="PSUM") as pA, \
                 tc.tile_pool(name="paT", bufs=1, space="PSUM") as paT, \
                 tc.tile_pool(name="pG", bufs=1, space="PSUM") as pG:

                def gru_block2(wp, aTs):
                    """GRU update for the 256 nodes of window pair wp
                    (windows 2wp, 2wp+1); aTs is a [128, 512] AP holding aT
                    for both windows: cols = kc*256 + wi2*128 + d."""
                    nb = wp * 256
                    hw_ = [hT0[:, nb:nb + 256], hT1[:, nb:nb + 256]]
                    psg = []
                    for fc in range(2):
                        ps = pG.tile([128, 1024], f32, tag=f"gg{fc}",
                                     name=f"gg{fc}")
                        psg.append(ps)
                        # cols: r 0:256 | z 256:512 | ig 512:768 | hg 768:1024
                        for gi in (0, 1):
                            col = gi * 256
                            mi = gi * 2 + fc
                            nc.tensor.matmul(ps[:, col:col + 256],
                                             lhsT=W(f"wi_{mi}_0"),
                                             rhs=aTs[:, 0:256],
                                             start=True, stop=False)
                            nc.tensor.matmul(ps[:, col:col + 256],
                                             lhsT=W(f"wi_{mi}_1"),
                                             rhs=aTs[:, 256:512],
                                             start=False, stop=False)
                            nc.tensor.matmul(ps[:, col:col + 256],
                                             lhsT=W(f"wh_{mi}_0"), rhs=hw_[0],
                                             start=False, stop=False)
                            nc.tensor.matmul(ps[:, col:col + 256],
                                             lhsT=W(f"wh_{mi}_1"), rhs=hw_[1],
                                             start=False, stop=True)
                        mi = 4 + fc
                        nc.tensor.matmul(ps[:, 512:768], lhsT=W(f"wi_{mi}_0"),
                                         rhs=aTs[:, 0:256], start=True, stop=False)
                        nc.tensor.matmul(ps[:, 512:768], lhsT=W(f"wi_{mi}_1"),
                                         rhs=aTs[:, 256:512], start=False, stop=True)
                        nc.tensor.matmul(ps[:, 768:1024], lhsT=W(f"wh_{mi}_0"),
                                         rhs=hw_[0], start=True, stop=False)
                        nc.tensor.matmul(ps[:, 768:1024], lhsT=W(f"wh_{mi}_1"),
                                         rhs=hw_[1], start=False, stop=True)
                    for fc in range(2):
                        ps = psg[fc]
                        hslice = hT[fc][:, nb:nb + 256]
                        r = gtp.tile([128, 256], bf, tag="r")
                        z = gtp.tile([128, 256], bf, tag="z")
                        t1 = gtp.tile([128, 256], f32, tag="t1")
                        g = gtp.tile([128, 256], f32, tag="g2")
                        d = gtp.tile([128, 256], f32, tag="d")
                        nc.scalar.activation(r[:], ps[:, 0:256], AF.Sigmoid,
                                             bias=Bias("br", fc))
                        nc.scalar.activation(z[:], ps[:, 256:512], AF.Sigmoid,
                                             bias=Bias("bz", fc))
                        # t1 = (hg + bhg) * r
                        nc.vector.scalar_tensor_tensor(
                            t1[:], ps[:, 768:1024], Bias("bhg", fc), r[:],
                            op0=ALU.add, op1=ALU.mult)
                        nc.vector.tensor_add(t1[:], t1[:], ps[:, 512:768])
                        nc.scalar.activation(g[:], t1[:], AF.Tanh,
                                             bias=Bias("big", fc))
                        nc.vector.tensor_sub(d[:], hslice, g[:])
                        nc.vector.tensor_mul(d[:], z[:], d[:])
                        nc.vector.tensor_add(hslice, g[:], d[:])

                # ---- step 0: aT precomputed on host ----
                for wp in range(NWIN // 2):
                    gru_block2(wp, aT0sb[:, wp * 512:(wp + 1) * 512])

                for s in range(1, nsteps):
                    in_cc = dpool.tile([NPC, OUT], tdt, name=f"in_cc{s}")
                    out_cc = dpool.tile([NN, OUT], tdt, addr_space="Shared",
                                        name=f"out_cc{s}")
                    # --- export local shard row-major: h[4096, 256] ---
                    for blk in range(NPC // 128):
                        pst = pT.tile([128, 256], bf, tag="pst")
                        nc.tensor.transpose(
                            pst[:, 0:128],
                            hT0[:, blk * 128:(blk + 1) * 128], W("ident"))
                        nc.tensor.transpose(
                            pst[:, 128:256],
                            hT1[:, blk * 128:(blk + 1) * 128], W("ident"))
                        stg = stp.tile([128, 256], tdt, tag="stg")
                        if blk % 2 == 0:
                            nc.vector.tensor_copy(stg[:], pst[:])
                        else:
                            nc.scalar.activation(stg[:], pst[:], AF.Copy)
                        nc.sync.dma_start(
                            in_cc[blk * 128:(blk + 1) * 128, :], stg[:])
                    # --- AllGather the raw node table ---
                    if sim_mode or "collective" in ablate:
                        nc.sync.dma_start(out_cc[0:NPC, :], in_cc[:])
                    else:
                        nc.gpsimd.collective_compute(
                            "AllGather", ALU.bypass,
                            replica_groups=[list(range(num_devices))],
                            ins=[in_cc.opt()],
                            outs=[out_cc.opt()],
                        )
                    # optionally bounce the table into local DRAM: random
                    # gather reads from the Shared collective region may be
                    # slower than from a Local allocation
                    if "localtab" in ablate:
                        loc_cc = dpool.tile([NN, OUT], tdt, name=f"loc{s}")
                        nc.sync.dma_start(loc_cc[:, :], out_cc[:, :])
                        tab = loc_cc
                    else:
                        tab = out_cc
                    # --- per window: gather, per-etype scatter, W-apply, GRU ---
                    for w in range(NWIN):
                        gts = []
                        for k in range(NCALL):
                            ncol = min(GCH, SCF - k * GCH)
                            if "gather" in ablate:
                                gts.append(gdum[0])
                                continue
                            gt = gap.tile([128, GCH, OUT], tdt, tag=f"g{k}",
                                          name=f"g{k}")
                            c0 = (w * SCF + k * GCH) * 8
                            nc.gpsimd.dma_gather(
                                gt[:, 0:ncol, :],
                                tab[:, :],
                                idx_sb[:, c0:c0 + ncol * 8],
                                ncol * 128,
                                ncol * 128,
                                OUT,
                                single_packet=single_packet,
                                queue_num=(w * NCALL + k) % nq)
                            gts.append(gt)
                        psA = [pA.tile([128, 512], f32, tag=f"A{fc}",
                                       name=f"A{fc}")
                               for fc in range(2)]
                        for fc in range(2):
                            for t in range(NT):
                                nf = 1 if "scatter" in ablate else F[t]
                                hastail = tl[t] and "scatter" not in ablate
                                nmm = nf + (1 if hastail else 0)
                                i = 0
                                j = 0
                                while j < nf:
                                    col = colof[t] + j
                                    scol = (w * SCF + col) * 128
                                    gt = gts[col // GCH]
                                    slot = col % GCH
                                    pair = (table_fp8 and j + 1 < nf
                                            and slot + 1 < GCH)
                                    if pair:
                                        nc.tensor.matmul(
                                            psA[fc][:, t * 128:(t + 1) * 128],
                                            lhsT=gt[:, slot:slot + 2,
                                                    fc * 128:(fc + 1) * 128],
                                            rhs=Ssb[:, scol:scol + 256].rearrange(
                                                "p (two n) -> p two n", two=2),
                                            start=(i == 0), stop=(i + 2 == nmm),
                                            perf_mode=mybir.MatmulPerfMode.DoubleRow)
                                        j += 2
                                        i += 2
                                    else:
                                        nc.tensor.matmul(
                                            psA[fc][:, t * 128:(t + 1) * 128],
                                            lhsT=gt[:, slot,
                                                    fc * 128:(fc + 1) * 128],
                                            rhs=Ssb[:, scol:scol + 128],
                                            start=(i == 0), stop=(i + 1 == nmm))
                                        j += 1
                                        i += 1
                                if hastail:
                                    tc_, toff = tailpos[t]
                                    scol = (w * SCF + tc_) * 128
                                    gt = gts[tc_ // GCH]
                                    slot = tc_ % GCH
                                    nc.tensor.matmul(
                                        psA[fc][:, t * 128:(t + 1) * 128],
                                        lhsT=gt[toff:toff + 64, slot,
                                                fc * 128:(fc + 1) * 128],
                                        rhs=Ssb[toff:toff + 64, scol:scol + 128],
                                        start=(i == 0), stop=True)
                        A_sb = asp.tile([128, 1024], bf, tag="Asb")
                        if w % 2 == 0:
                            nc.vector.tensor_copy(A_sb[:, 0:512], psA[0][:])
                            nc.scalar.activation(A_sb[:, 512:1024], psA[1][:], AF.Copy)
                        else:
                            nc.scalar.activation(A_sb[:, 0:512], psA[0][:], AF.Copy)
                            nc.vector.tensor_copy(A_sb[:, 512:1024], psA[1][:])
                        # aT = sum_t W_msg[t]^T @ A_t  (transposed layout)
                        pa = paT.tile([128, 256], f32, tag="aT")
                        for mo in range(2):
                            nmm = 0
                            for t in range(NT):
                                for kc in range(2):
                                    nc.tensor.matmul(
                                        pa[:, mo * 128:(mo + 1) * 128],
                                        lhsT=W(f"wmsg_{t}_{kc}_{mo}"),
                                        rhs=A_sb[:, kc * 512 + t * 128:
                                                 kc * 512 + t * 128 + 128],
                                        start=(nmm == 0), stop=(nmm == 7))
                                    nmm += 1
                        if w % 2 == 0:
                            aTs2 = asp.tile([128, 512], bf, tag="aTs",
                                            name="aTs2")
                        wi2 = w % 2
                        if w % 2 == 0:
                            nc.vector.tensor_copy(
                                aTs2[:, wi2 * 128:wi2 * 128 + 128],
                                pa[:, 0:128])
                            nc.vector.tensor_copy(
                                aTs2[:, 256 + wi2 * 128:256 + wi2 * 128 + 128],
                                pa[:, 128:256])
                        else:
                            nc.scalar.activation(
                                aTs2[:, wi2 * 128:wi2 * 128 + 128],
                                pa[:, 0:128], AF.Copy)
                            nc.scalar.activation(
                                aTs2[:, 256 + wi2 * 128:256 + wi2 * 128 + 128],
                                pa[:, 128:256], AF.Copy)
                        if w % 2 == 1:
                            gru_block2(w // 2, aTs2[:])

            # ---------------- readout ----------------
            with tc.tile_pool(name="rsb", bufs=3) as rsb, \
                 tc.tile_pool(name="rx", bufs=1) as rxp, \
                 tc.tile_pool(name="pR", bufs=1, space="PSUM") as pR, \
                 tc.tile_pool(name="pV", bufs=1, space="PSUM") as pV:
                res_sb = pp.tile([1, GPC], f32)
                xTb = rxp.tile([128, NPC], bf)
                nc.sync.dma_start(xTb[:], xT_d.ap())
                if "readout" in ablate:
                    nc.gpsimd.memset(res_sb[:], 0.0)
                    nc.vector.tensor_copy(res_sb[0:1, 0:1], hT0[0:1, 0:1])
                    nc.sync.dma_start(out_d.ap(), res_sb[0:1, :])

                def pool_step(y, width, ksz, tag):
                    # maxpool stride 2 over free dim, VALID
                    outw = (width - ksz) // 2 + 1
                    pout = rsb.tile([128, outw], bf, tag=tag)
                    ab = y[:, 0:2 * outw].rearrange("p (n t) -> p n t", t=2)
                    a0 = ab[:, :, 0]
                    a1 = ab[:, :, 1]
                    if ksz == 3:
                        tmp = rsb.tile([128, outw], bf, tag=tag + "_t")
                        nc.vector.tensor_max(tmp[:], a0, a1)
                        a2 = y[:, 2:2 * outw + 2].rearrange("p (n t) -> p n t", t=2)[:, :, 0]
                        nc.vector.tensor_max(pout[:], tmp[:], a2)
                    else:
                        nc.vector.tensor_max(pout[:], a0, a1)
                    return pout

                for gidx in (range(0) if "readout" in ablate else range(GPC)):
                    g0 = gidx * LG
                    # ---- Y path (h only, 256 ch) ----
                    y2p = []
                    for mo in range(2):
                        psY = pR.tile([128, 254], f32, space="PSUM", tag="psY")
                        n_mm = 0
                        for tap in range(3):
                            for kc in range(2):
                                nc.tensor.matmul(
                                    psY[:],
                                    lhsT=W(f"c1_{tap}_{kc}_{mo}"),
                                    rhs=hT[kc][:, g0 + tap:g0 + tap + 254],
                                    start=(n_mm == 0), stop=(n_mm == 5))
                                n_mm += 1
                        y1 = rsb.tile([128, 254], bf, tag=f"y1_{mo}")
                        nc.scalar.activation(y1[:], psY[:], AF.Relu,
                                             bias=Bias("c1b", mo))
                        y2p.append(pool_step(y1, 254, 3, f"p3_{mo}"))
                    y3p = []
                    for mo in range(2):
                        psY2 = pR.tile([128, 126], f32, space="PSUM", tag="psY2")
                        for kc in range(2):
                            nc.tensor.matmul(psY2[:], lhsT=W(f"c2_{kc}_{mo}"),
                                             rhs=y2p[kc][:],
                                             start=(kc == 0), stop=(kc == 1))
                        y2 = rsb.tile([128, 126], bf, tag=f"y2_{mo}")
                        nc.scalar.activation(y2[:], psY2[:], AF.Relu,
                                             bias=Bias("c2b", mo))
                        y3p.append(pool_step(y2, 126, 2, f"yp_{mo}"))
                    psy = pV.tile([1, 63], f32, space="PSUM", tag="psy")
                    for kc in range(2):
                        nc.tensor.matmul(psy[:], lhsT=W(f"wy_{kc}"),
                                         rhs=y3p[kc][:],
                                         start=(kc == 0), stop=(kc == 1))
                    # ---- Z path (concat h|x, 384 ch) ----
                    cch = [hT[0], hT[1], xTb]
                    z2p = []
                    for mo in range(3):
                        psZ = pR.tile([128, 254], f32, space="PSUM", tag="psZ")
                        n_mm = 0
                        for tap in range(3):
                            for kc in range(3):
                                nc.tensor.matmul(
                                    psZ[:],
                                    lhsT=W(f"cc1_{tap}_{kc}_{mo}"),
                                    rhs=cch[kc][:, g0 + tap:g0 + tap + 254],
                                    start=(n_mm == 0), stop=(n_mm == 8))
                                n_mm += 1
                        z1 = rsb.tile([128, 254], bf, tag=f"z1_{mo}")
                        nc.scalar.activation(z1[:], psZ[:], AF.Relu,
                                             bias=Bias("cc1b", mo))
                        z2p.append(pool_step(z1, 254, 3, f"zp_{mo}"))
                    z3p = []
                    for mo in range(3):
                        psZ2 = pR.tile([128, 126], f32, space="PSUM", tag="psZ2")
                        for kc in range(3):
                            nc.tensor.matmul(psZ2[:], lhsT=W(f"cc2_{kc}_{mo}"),
                                             rhs=z2p[kc][:],
                                             start=(kc == 0), stop=(kc == 2))
                        z2 = rsb.tile([128, 126], bf, tag=f"z2_{mo}")
                        nc.scalar.activation(z2[:], psZ2[:], AF.Relu,
                                             bias=Bias("cc2b", mo))
                        z3p.append(pool_step(z2, 126, 2, f"zq_{mo}"))
                    psz = pV.tile([1, 63], f32, space="PSUM", tag="psz")
                    for kc in range(3):
                        nc.tensor.matmul(psz[:], lhsT=W(f"wz_{kc}"),
                                         rhs=z3p[kc][:],
                                         start=(kc == 0), stop=(kc == 2))
                    # ---- combine ----
                    ty = rsb.tile([1, 63], f32, tag="ty")
                    tz = rsb.tile([1, 63], f32, tag="tz")
                    pr = rsb.tile([1, 63], f32, tag="pr")
                    sm = rsb.tile([1, 1], f32, tag="sm")
                    nc.vector.tensor_scalar_add(ty[:], psy[:], Bias1("by"))
                    nc.vector.tensor_scalar_add(tz[:], psz[:], Bias1("bz_"))
                    nc.vector.tensor_mul(pr[:], ty[:], tz[:])
                    nc.vector.tensor_reduce(sm[:], pr[:],
                                            axis=mybir.AxisListType.X, op=ALU.add)
                    nc.scalar.activation(res_sb[0:1, gidx:gidx + 1], sm[:],
                                         AF.Sigmoid, scale=1.0 / 63.0)
                if "readout" not in ablate:
                    nc.sync.dma_start(out_d.ap(), res_sb[0:1, :])
    nc.finalize()
    return nc


# ---------------------------------------------------------------------------
# host entry
# ---------------------------------------------------------------------------
def _prepare(inputs):
    features = np.asarray(inputs["features"], np.float32)
    src = np.asarray(inputs["src"]).astype(np.int64)
    dst = np.asarray(inputs["dst"]).astype(np.int64)
    etype = np.asarray(inputs["etype"]).astype(np.int64)
    W_msg = np.asarray(inputs["W_msg"], np.float32)
    wl = _make_wlayout()
    bl = _make_blayout()
    Wimg = _pack_weights(
        wl,
        W_msg,
        np.asarray(inputs["gru_Wi"], np.float32),
        np.asarray(inputs["gru_Wh"], np.float32),
        np.asarray(inputs["conv1_w"], np.float32),
        np.asarray(inputs["conv2_w"], np.float32),
        np.asarray(inputs["convc1_w"], np.float32),
        np.asarray(inputs["convc2_w"], np.float32),
        np.asarray(inputs["wy"], np.float32),
        np.asarray(inputs["wz"], np.float32),
    )
    Bimg = _pack_biases(
        bl,
        np.asarray(inputs["gru_bi"], np.float32),
        np.asarray(inputs["gru_bh"], np.float32),
        np.asarray(inputs["conv1_b"], np.float32),
        np.asarray(inputs["conv2_b"], np.float32),
        np.asarray(inputs["convc1_b"], np.float32),
        np.asarray(inputs["convc2_b"], np.float32),
        np.asarray(inputs["by"], np.float32),
        np.asarray(inputs["bz"], np.float32),
    )
    # b_msg is folded into nothing here -- reference setup has zeros.
    b_msg = np.asarray(inputs["b_msg"], np.float32)
    assert np.abs(b_msg).max() == 0.0, "nonzero b_msg not supported"

    idx16, S_img, C64, SCF = _preprocess_edges(src, dst, etype)
    aT0 = _precompute_step0(features, src, dst, etype, W_msg)

    in_maps = []
    for c in range(NCORES):
        feats = features[c * NPC:(c + 1) * NPC]  # [4096, 128]
        xT = feats.T.astype(BF16)                # [128, 4096]
        h0T = np.zeros((OUT, NPC), np.float32)
        h0T[:IN_DIM] = feats.T
        im = {
            "h0T": h0T.astype(BF16),
            "xT": xT,
            "aT0": aT0[c],
            "Wimg": Wimg,
            "Bimg": Bimg,
            "Simg": S_img[c],
            "idx": idx16[c],
        }
        in_maps.append(im)
    return wl, bl, C64, in_maps


def kernel(**inputs):
    wl, bl, C64, in_maps = _prepare(inputs)
    nc = build_program(C64, wl, bl)
    res = run_bass_kernel_spmd(nc, in_maps, core_ids=list(range(NCORES)))
    out = np.concatenate([res.results[c]["out"] for c in range(NCORES)])
    return out.astype(np.float32)
